# revision 33
# baseline (speedup 1.0000x reference)
"""Trainium2 Bass kernel: basic GCN layer, row-parallel over 8 NeuronCores.

    Y = relu( D^-1/2 (A + I) D^-1/2 (H @ W.T + b) ),  D = (A + I).sum(axis=1)

Sharding: core i owns output rows [i*1024, (i+1)*1024).  Each core receives
(A + I)[rows, :].T pre-tiled host-side into [128, 64*1024] fp8 so every DMA
descriptor moves an 8 KiB contiguous line (A+I is 0/1/2 — fp8 is lossless).
A stays fp8 in SBUF; matmuls use it as the moving operand against bf16
stationaries (mixed-dtype matmul is legal on TRN2).

Schedule per core (the first collective cannot start before the ~40-65 us
entry barrier + ~11 us ncfw setup, so everything before it is free time):
  - A loads in 8 chunks balanced over the gpsimd/sync/scalar DMA rings
    (~105 GB/s each); the PE runs the row-sum matmuls paced with the
    arriving chunks, as fp8 DoubleRow matmuls: adjacent k-tile pairs ride
    the two weight planes (all-ones), streaming A at 2 elements/cycle.
  - H.T follows on the same rings; HW = H @ W.T + b is computed in
    4-k-tile batches per PSUM bank, copied to SBUF by the scalar engine
    (keeps the vector queue clear and avoids per-tile ping-pong stalls).
  - Row sums go out in ONE 8-rank AllGather (4 KiB), triggered right
    after the row-sum matmuls finish (~1.5 us past the last A byte).
  - When the AG lands: gathered sums -> PE transpose -> dinv = 1/sqrt on
    [128, 64]; the 64 per-k-tile scales of HW alternate vector/scalar,
    racing ahead of the main matmuls (X^T A-tile into two PSUM halves);
    epilogue relu(dl * main); DMA out per half on separate rings.
"""

import os
import sys

import numpy as np

for _p in ("/opt/trn_rl_repo", "/root/.axon_site/_ro/trn_rl_repo"):
    if _p not in sys.path and os.path.isdir(_p):
        sys.path.insert(0, _p)

N = 8192        # nodes
NCORES = 8
RPC = N // NCORES  # rows per core (1024)
P = 128         # partitions / tile edge
F = 128         # feature dim (in == out)


def _build_nc(n=8192, rpc=1024, f=128, ncores=8):
    import concourse.bass as bass  # noqa: F401
    import concourse.mybir as mybir
    from concourse import bacc, tile
    from concourse.masks import make_identity

    dt = mybir.dt
    f32, bf, f8 = dt.float32, dt.bfloat16, dt.float8e4

    P = 128
    kt = n // P                 # contraction tiles (64)
    NCH = 8                     # A DMA chunks
    kpc = kt // NCH             # k-tiles per chunk (8)
    RC = 512                    # PSUM half width
    NRC = rpc // RC             # 2 halves
    KB = 4                      # k-tiles batched per hw-pre PSUM bank
    NB = kt // KB               # 16 hw-pre batches

    nc = bacc.Bacc("TRN2", num_devices=ncores)

    at = nc.dram_tensor("at", [P, kt * rpc], f8, kind="ExternalInput")   # (A+I)[rows].T pre-tiled
    ht = nc.dram_tensor("ht", [f, n], bf, kind="ExternalInput")          # H.T
    wt = nc.dram_tensor("wt", [f, f], bf, kind="ExternalInput")          # W.T
    bias = nc.dram_tensor("bias", [1, f], bf, kind="ExternalInput")      # b
    out = nc.dram_tensor("out", [f, rpc], bf, kind="ExternalOutput")     # Y[rows].T

    with tile.TileContext(nc) as tc:
        with (
            tc.tile_pool(name="const", bufs=1) as cpool,
            tc.tile_pool(name="abuf", bufs=1) as apool,
            tc.tile_pool(name="work", bufs=1) as wpool,
            tc.tile_pool(name="pshw", bufs=3, space="PSUM") as pshw,
            tc.tile_pool(name="psbig", bufs=1, space="PSUM") as psbig,
            tc.tile_pool(name="dram", bufs=1, space="DRAM") as dpool,
        ):
            # ---- constants / small inputs ----
            wt_sb = cpool.tile([f, f], bf, tag="wt", name="wt_sb")
            bias_sb = cpool.tile([1, f], bf, tag="bias", name="bias_sb")
            ones2 = cpool.tile([P, 2 * P], f8, tag="ones2", name="ones2")
            ones_r = cpool.tile([1, P], bf, tag="onesr", name="ones_r")
            ident = cpool.tile([P, P], f32, tag="ident", name="ident")
            nc.vector.memset(ones2[:], 1.0)
            nc.vector.memset(ones_r[:], 1.0)
            make_identity(nc, ident[:])
            nc.scalar.dma_start(wt_sb[:], wt[:])
            nc.scalar.dma_start(bias_sb[:], bias[:])

            # ---- A over three DMA rings in few big transfers ----
            # gp: chunks 0-1, 2 | sync: chunks 3-4, 5 | scalar: 6, 7
            # (scalar also carries the tiny ci upload so the AG trigger
            # never queues behind H.T).
            a_all = apool.tile([P, kt * rpc], f8, tag="a", name="a_all")
            ht_all = cpool.tile([f, n], bf, tag="ht", name="ht_all")
            W8 = kpc * rpc                      # bytes per chunk row-block

            def a_dma(eng, c0, c1):
                eng.dma_start(a_all[:, c0 * W8:c1 * W8],
                              at[:, c0 * W8:c1 * W8])

            a_dma(nc.gpsimd, 0, 1)
            a_dma(nc.sync, 4, 5)
            a_dma(nc.gpsimd, 1, 3)
            a_dma(nc.sync, 5, 7)
            a_dma(nc.gpsimd, 3, 4)
            a_dma(nc.sync, 7, 8)
            # H.T rides the slow-ramp scalar ring: it is only consumed by
            # the hw-pre matmuls after the row sums, well past its landing
            nc.scalar.dma_start(ht_all[:, 0:n // 2], ht[:, 0:n // 2])
            nc.scalar.dma_start(ht_all[:, n // 2:], ht[:, n // 2:])
            a_ch = [a_all[:, c * W8:(c + 1) * W8] for c in range(NCH)]
            ht_ch = [ht_all[:, c * rpc:(c + 1) * rpc] for c in range(NCH)]

            # chunk layout is h-major: col = h*(kpc*RC) + (k%kpc)*RC + i
            def a_slice(k, h):
                base = h * kpc * RC + (k % kpc) * RC
                return a_ch[k // kpc][:, base:base + RC]

            # ---- row sums, paced with the A chunks ----
            # fp8 DoubleRow: adjacent k-tile pairs ride the two weight
            # planes (all-ones), streaming A at 2 elements/cycle.
            ps_rs = [psbig.tile([P, RC], f32, tag=f"rs{h}", name=f"rs{h}")
                     for h in range(NRC)]
            ones2_ap = ones2[:].rearrange("p (o m) -> p o m", o=2)
            ppc = kpc // 2                      # pairs per chunk
            chunk_order = [0, 4, 1, 2, 5, 6, 3, 7]   # expected arrival order
            for ci_, c in enumerate(chunk_order):
                for tl in range(ppc):
                    for h in range(NRC):
                        base = h * kpc * RC + tl * 2 * RC
                        pair = a_ch[c][:, base:base + 2 * RC].rearrange(
                            "p (o i) -> p o i", o=2)
                        nc.tensor.matmul(
                            ps_rs[h][:, :], ones2_ap, pair,
                            start=(ci_ == 0 and tl == 0),
                            stop=(ci_ == NCH - 1 and tl == ppc - 1),
                            perf_mode=mybir.MatmulPerfMode.DoubleRow)

            # row sums -> SBUF (vector engine) -> DRAM -> AllGather
            rs_sb = wpool.tile([1, rpc], f32, tag="rs_sb", name="rs_sb")
            for h in range(NRC):
                nc.vector.tensor_copy(rs_sb[0:1, h * RC:(h + 1) * RC],
                                      ps_rs[h][0:1, :])
            ci = dpool.tile([1, rpc], f32, tag="ccin", name="cc_in")
            co = dpool.tile([ncores, rpc], f32, tag="ccout", name="cc_out",
                            addr_space="Shared")
            nc.scalar.dma_start(ci[:], rs_sb[:])
            nc.gpsimd.collective_compute(
                "AllGather", mybir.AluOpType.bypass,
                replica_groups=[list(range(ncores))],
                ins=[ci.opt()], outs=[co.opt()],
            )

            # ---- HW = H @ W.T + b, 4 k-tiles per PSUM bank ----
            hw_sb = wpool.tile([P, n], bf, tag="hw", name="hw_sb")
            for g in range(NB):
                ps4 = pshw.tile([P, KB * f], f32, tag="hw4", name=f"hw4_{g}")
                for m in range(KB):
                    k = g * KB + m
                    sl = ps4[:, m * f:(m + 1) * f]
                    nc.tensor.matmul(sl,
                                     ht_ch[k // kpc][:, (k % kpc) * P:
                                                     (k % kpc + 1) * P],
                                     wt_sb[:, :], start=True, stop=False)
                    nc.tensor.matmul(sl, ones_r[0:1, 0:P],
                                     bias_sb[0:1, :], start=False, stop=True)
                nc.scalar.copy(hw_sb[:, g * KB * f:(g + 1) * KB * f], ps4[:, :])

            # ---- gathered sums -> dinv[p, k] = 1/sqrt(s[128k + p]) ----
            rs2d = wpool.tile([kt, P], f32, tag="rs2d", name="rs2d")
            nc.sync.dma_start(
                rs2d[:], co[:].rearrange("g (m p) -> (g m) p", p=P))
            ps_t = pshw.tile([P, kt], f32, tag="hw4", name="ps_t")
            nc.tensor.transpose(ps_t[:, :], rs2d[:, :], ident[0:kt, 0:kt])
            dinv = wpool.tile([P, kt], f32, tag="dinv", name="dinv")
            nc.scalar.sqrt(dinv[:, :], ps_t[:, :])
            nc.vector.reciprocal(dinv[:, :], dinv[:, :])

            # ---- dl broadcast for the epilogue: dlb[p, i] = D_i^-1/2 ----
            # computed during the AG wait (only needs the local sums)
            dlb = wpool.tile([P, rpc], f32, tag="dlb", name="dlb")
            nc.sync.dma_start(
                dlb[:].rearrange("p (o r) -> p o r", o=1),
                ci[0:1, :].partition_broadcast(P),
            )
            nc.scalar.sqrt(dlb[:, :], dlb[:, :])
            nc.vector.reciprocal(dlb[:, :], dlb[:, :])

            # ---- scale + main matmuls: ps_main[h] += (dinv_k*HW_k)^T A_k ----
            # h-major: ps_main[0] finishes halfway so its epilogue and
            # output DMA overlap the h=1 matmuls.  The per-k scales
            # alternate vector/scalar (h=0 pass only) so either engine
            # only has to keep half pace with the PE.
            ps_main = [psbig.tile([f, RC], f32, tag=f"main{h}", name=f"main{h}")
                       for h in range(NRC)]
            y_sb = wpool.tile([f, rpc], bf, tag="y", name="y_sb")
            QC = RC // 2

            def epilogue(h):
                for m in range(2):
                    lo = h * RC + m * QC
                    sl = y_sb[:, lo:lo + QC]
                    nc.vector.tensor_mul(sl,
                                         ps_main[h][:, m * QC:(m + 1) * QC],
                                         dlb[:, lo:lo + QC])
                    nc.vector.tensor_scalar_max(sl, sl, 0.0)
                    eng = nc.gpsimd if m == 0 else nc.sync
                    eng.dma_start(out[:, lo:lo + QC], sl)

            for h in range(NRC):
                for k in range(kt):
                    sl = hw_sb[:, k * f:(k + 1) * f]
                    if h == 0:
                        if k % 2 == 0:
                            nc.vector.tensor_scalar_mul(sl, sl,
                                                        dinv[:, k:k + 1])
                        else:
                            nc.scalar.mul(sl, sl, dinv[:, k:k + 1])
                    nc.tensor.matmul(
                        ps_main[h][:, :], sl, a_slice(k, h),
                        start=(k == 0), stop=(k == kt - 1),
                    )
                epilogue(h)

    nc.compile()
    return nc


_CACHE = {}


def _get_nc():
    if "nc" not in _CACHE:
        _CACHE["nc"] = _build_nc()
    return _CACHE["nc"]


def _prep_in_maps(H, A, W, b):
    import ml_dtypes

    bf16 = ml_dtypes.bfloat16
    H = np.asarray(H, dtype=np.float32)
    A = np.asarray(A, dtype=np.float32)
    W = np.asarray(W, dtype=np.float32)
    b = np.asarray(b, dtype=np.float32)
    ht = np.ascontiguousarray(H.T.astype(bf16))
    wt = np.ascontiguousarray(W.T.astype(bf16))
    bias = np.ascontiguousarray(b.reshape(1, -1).astype(bf16))
    idx = np.arange(RPC)
    maps = []
    for i in range(NCORES):
        rows = slice(i * RPC, (i + 1) * RPC)
        Asl = A[rows, :].copy()
        Asl[idx, i * RPC + idx] += 1.0          # fold in A + I (0/1/2: exact)
        # pre-tile (A+I)[rows].T -> [128, kt*rpc], chunks of 8 k-tiles with
        # h-major columns: col = c*8192 + h*4096 + (k%8)*512 + i
        at = (Asl.T.reshape(8, 8, P, 2, 512)    # [c, kl, p, h, i]
              .transpose(2, 0, 3, 1, 4).reshape(P, -1))
        maps.append({
            "at": np.ascontiguousarray(at.astype(ml_dtypes.float8_e4m3)),
            "ht": ht,
            "wt": wt,
            "bias": bias,
        })
    return maps


def run(H, A, W, b, trace=False):
    from concourse import bass_utils

    nc = _get_nc()
    res = bass_utils.run_bass_kernel_spmd(
        nc, _prep_in_maps(H, A, W, b), core_ids=list(range(NCORES)),
        trace=trace,
    )
    Y = np.concatenate(
        [np.asarray(res.results[i]["out"]).T for i in range(NCORES)], axis=0
    )
    return np.ascontiguousarray(Y, dtype=np.float32), res


def kernel(H, A, W, b):
    return run(H, A, W, b)[0]


# revision 37
# speedup vs baseline: 1.0230x; 1.0230x over previous
"""Trainium2 Bass kernel: basic GCN layer, row-parallel over 8 NeuronCores.

    Y = relu( D^-1/2 (A + I) D^-1/2 (H @ W.T + b) ),  D = (A + I).sum(axis=1)

Sharding: core i owns output rows [i*1024, (i+1)*1024).  Each core receives
(A + I)[rows, :].T pre-tiled host-side into [128, 64*1024] fp8 so every DMA
descriptor moves an 8 KiB contiguous line (A+I is 0/1/2 — fp8 is lossless).
A stays fp8 in SBUF; matmuls use it as the moving operand against bf16
stationaries (mixed-dtype matmul is legal on TRN2).

Schedule per core (the first collective cannot start before the ~40-65 us
entry barrier + ~11 us ncfw setup, so everything before it is free time):
  - A loads in 8 chunks balanced over the gpsimd/sync/scalar DMA rings
    (~105 GB/s each); the PE runs the row-sum matmuls paced with the
    arriving chunks, as fp8 DoubleRow matmuls: adjacent k-tile pairs ride
    the two weight planes (all-ones), streaming A at 2 elements/cycle.
  - H.T follows on the same rings; HW = H @ W.T + b is computed in
    4-k-tile batches per PSUM bank, copied to SBUF by the scalar engine
    (keeps the vector queue clear and avoids per-tile ping-pong stalls).
  - Row sums go out in ONE 8-rank AllGather (4 KiB), triggered right
    after the row-sum matmuls finish (~1.5 us past the last A byte).
  - When the AG lands: gathered sums -> PE transpose -> dinv = 1/sqrt on
    [128, 64]; the 64 per-k-tile scales of HW alternate vector/scalar,
    racing ahead of the main matmuls (X^T A-tile into two PSUM halves);
    epilogue relu(dl * main); DMA out per half on separate rings.
"""

import os
import sys

import numpy as np

for _p in ("/opt/trn_rl_repo", "/root/.axon_site/_ro/trn_rl_repo"):
    if _p not in sys.path and os.path.isdir(_p):
        sys.path.insert(0, _p)

N = 8192        # nodes
NCORES = 8
RPC = N // NCORES  # rows per core (1024)
P = 128         # partitions / tile edge
F = 128         # feature dim (in == out)


def _build_nc(n=8192, rpc=1024, f=128, ncores=8):
    import concourse.bass as bass  # noqa: F401
    import concourse.mybir as mybir
    from concourse import bacc, tile
    from concourse.masks import make_identity

    dt = mybir.dt
    f32, bf, f8 = dt.float32, dt.bfloat16, dt.float8e4

    P = 128
    kt = n // P                 # contraction tiles (64)
    NCH = 8                     # A DMA chunks
    kpc = kt // NCH             # k-tiles per chunk (8)
    RC = 512                    # PSUM half width
    NRC = rpc // RC             # 2 halves
    KB = 4                      # k-tiles batched per hw-pre PSUM bank
    NB = kt // KB               # 16 hw-pre batches

    nc = bacc.Bacc("TRN2", num_devices=ncores)

    at = nc.dram_tensor("at", [P, kt * rpc], f8, kind="ExternalInput")   # (A+I)[rows].T pre-tiled
    ht = nc.dram_tensor("ht", [f, n], bf, kind="ExternalInput")          # H.T
    wt = nc.dram_tensor("wt", [f, f], bf, kind="ExternalInput")          # W.T
    bias = nc.dram_tensor("bias", [1, f], bf, kind="ExternalInput")      # b
    out = nc.dram_tensor("out", [f, rpc], bf, kind="ExternalOutput")     # Y[rows].T

    with tile.TileContext(nc) as tc:
        with (
            tc.tile_pool(name="const", bufs=1) as cpool,
            tc.tile_pool(name="abuf", bufs=1) as apool,
            tc.tile_pool(name="work", bufs=1) as wpool,
            tc.tile_pool(name="pshw", bufs=3, space="PSUM") as pshw,
            tc.tile_pool(name="psbig", bufs=1, space="PSUM") as psbig,
            tc.tile_pool(name="dram", bufs=1, space="DRAM") as dpool,
        ):
            # ---- constants / small inputs ----
            wt_sb = cpool.tile([f, f], bf, tag="wt", name="wt_sb")
            bias_sb = cpool.tile([1, f], bf, tag="bias", name="bias_sb")
            ones2 = cpool.tile([P, 2 * P], f8, tag="ones2", name="ones2")
            ones_r = cpool.tile([1, P], bf, tag="onesr", name="ones_r")
            ident = cpool.tile([P, P], f32, tag="ident", name="ident")
            nc.vector.memset(ones2[:], 1.0)
            nc.vector.memset(ones_r[:], 1.0)
            make_identity(nc, ident[:])
            nc.scalar.dma_start(wt_sb[:], wt[:])
            nc.scalar.dma_start(bias_sb[:], bias[:])

            # ---- A over three DMA rings in few big transfers ----
            # gp: chunks 0-1, 2 | sync: chunks 3-4, 5 | scalar: 6, 7
            # (scalar also carries the tiny ci upload so the AG trigger
            # never queues behind H.T).
            a_all = apool.tile([P, kt * rpc], f8, tag="a", name="a_all")
            ht_all = cpool.tile([f, n], bf, tag="ht", name="ht_all")
            W8 = kpc * rpc                      # bytes per chunk row-block

            def a_dma(eng, c, parts=1):
                for q in range(parts):
                    lo = c * W8 + q * W8 // parts
                    hi = c * W8 + (q + 1) * W8 // parts
                    eng.dma_start(a_all[:, lo:hi], at[:, lo:hi])

            ring_of = [nc.gpsimd, nc.sync, nc.scalar, nc.gpsimd,
                       nc.sync, nc.scalar, nc.gpsimd, nc.sync]
            for c in range(NCH):
                a_dma(ring_of[c], c, parts=2 if c < 3 else 1)
            ht_ring = [nc.gpsimd, nc.sync, nc.scalar, nc.gpsimd,
                       nc.sync, nc.scalar, nc.gpsimd, nc.gpsimd]
            for c in range(NCH):
                ht_ring[c].dma_start(ht_all[:, c * rpc:(c + 1) * rpc],
                                     ht[:, c * rpc:(c + 1) * rpc])
            a_ch = [a_all[:, c * W8:(c + 1) * W8] for c in range(NCH)]
            ht_ch = [ht_all[:, c * rpc:(c + 1) * rpc] for c in range(NCH)]

            # chunk layout is h-major: col = h*(kpc*RC) + (k%kpc)*RC + i
            def a_slice(k, h):
                base = h * kpc * RC + (k % kpc) * RC
                return a_ch[k // kpc][:, base:base + RC]

            # ---- row sums, paced with the A chunks ----
            # fp8 DoubleRow: adjacent k-tile pairs ride the two weight
            # planes (all-ones), streaming A at 2 elements/cycle.
            ps_rs = [psbig.tile([P, RC], f32, tag=f"rs{h}", name=f"rs{h}")
                     for h in range(NRC)]
            ones2_ap = ones2[:].rearrange("p (o m) -> p o m", o=2)
            ppc = kpc // 2                      # pairs per chunk
            chunk_order = list(range(NCH))      # matches 3-ring arrival
            for ci_, c in enumerate(chunk_order):
                for tl in range(ppc):
                    for h in range(NRC):
                        base = h * kpc * RC + tl * 2 * RC
                        pair = a_ch[c][:, base:base + 2 * RC].rearrange(
                            "p (o i) -> p o i", o=2)
                        nc.tensor.matmul(
                            ps_rs[h][:, :], ones2_ap, pair,
                            start=(ci_ == 0 and tl == 0),
                            stop=(ci_ == NCH - 1 and tl == ppc - 1),
                            perf_mode=mybir.MatmulPerfMode.DoubleRow)

            # row sums -> SBUF (vector engine) -> DRAM -> AllGather
            rs_sb = wpool.tile([1, rpc], f32, tag="rs_sb", name="rs_sb")
            for h in range(NRC):
                nc.vector.tensor_copy(rs_sb[0:1, h * RC:(h + 1) * RC],
                                      ps_rs[h][0:1, :])
            ci = dpool.tile([1, rpc], f32, tag="ccin", name="cc_in")
            co = dpool.tile([ncores, rpc], f32, tag="ccout", name="cc_out",
                            addr_space="Shared")
            nc.gpsimd.dma_start(ci[:], rs_sb[:])
            nc.gpsimd.collective_compute(
                "AllGather", mybir.AluOpType.bypass,
                replica_groups=[list(range(ncores))],
                ins=[ci.opt()], outs=[co.opt()],
            )

            # ---- HW = H @ W.T + b, 4 k-tiles per PSUM bank ----
            hw_sb = wpool.tile([P, n], bf, tag="hw", name="hw_sb")
            for g in range(NB):
                ps4 = pshw.tile([P, KB * f], f32, tag="hw4", name=f"hw4_{g}")
                for m in range(KB):
                    k = g * KB + m
                    sl = ps4[:, m * f:(m + 1) * f]
                    nc.tensor.matmul(sl,
                                     ht_ch[k // kpc][:, (k % kpc) * P:
                                                     (k % kpc + 1) * P],
                                     wt_sb[:, :], start=True, stop=False)
                    nc.tensor.matmul(sl, ones_r[0:1, 0:P],
                                     bias_sb[0:1, :], start=False, stop=True)
                nc.scalar.copy(hw_sb[:, g * KB * f:(g + 1) * KB * f], ps4[:, :])

            # ---- gathered sums -> dinv[p, k] = 1/sqrt(s[128k + p]) ----
            rs2d = wpool.tile([kt, P], f32, tag="rs2d", name="rs2d")
            nc.sync.dma_start(
                rs2d[:], co[:].rearrange("g (m p) -> (g m) p", p=P))
            ps_t = pshw.tile([P, kt], f32, tag="hw4", name="ps_t")
            nc.tensor.transpose(ps_t[:, :], rs2d[:, :], ident[0:kt, 0:kt])
            dinv = wpool.tile([P, kt], f32, tag="dinv", name="dinv")
            nc.scalar.sqrt(dinv[:, :], ps_t[:, :])
            nc.vector.reciprocal(dinv[:, :], dinv[:, :])

            # ---- dl broadcast for the epilogue: dlb[p, i] = D_i^-1/2 ----
            # computed during the AG wait (only needs the local sums)
            dlb = wpool.tile([P, rpc], f32, tag="dlb", name="dlb")
            nc.sync.dma_start(
                dlb[:].rearrange("p (o r) -> p o r", o=1),
                ci[0:1, :].partition_broadcast(P),
            )
            nc.scalar.sqrt(dlb[:, :], dlb[:, :])
            nc.vector.reciprocal(dlb[:, :], dlb[:, :])

            # ---- scale + main matmuls: ps_main[h] += (dinv_k*HW_k)^T A_k ----
            # h-major: ps_main[0] finishes halfway so its epilogue and
            # output DMA overlap the h=1 matmuls.  The per-k scales
            # alternate vector/scalar (h=0 pass only) so either engine
            # only has to keep half pace with the PE.
            ps_main = [psbig.tile([f, RC], f32, tag=f"main{h}", name=f"main{h}")
                       for h in range(NRC)]
            y_sb = wpool.tile([f, rpc], bf, tag="y", name="y_sb")
            QC = RC // 2

            def epilogue(h):
                for m in range(2):
                    lo = h * RC + m * QC
                    sl = y_sb[:, lo:lo + QC]
                    nc.vector.tensor_mul(sl,
                                         ps_main[h][:, m * QC:(m + 1) * QC],
                                         dlb[:, lo:lo + QC])
                    nc.vector.tensor_scalar_max(sl, sl, 0.0)
                    eng = nc.gpsimd if m == 0 else nc.sync
                    eng.dma_start(out[:, lo:lo + QC], sl)

            for h in range(NRC):
                for k in range(kt):
                    sl = hw_sb[:, k * f:(k + 1) * f]
                    if h == 0:
                        if k % 2 == 0:
                            nc.vector.tensor_scalar_mul(sl, sl,
                                                        dinv[:, k:k + 1])
                        else:
                            nc.scalar.mul(sl, sl, dinv[:, k:k + 1])
                    nc.tensor.matmul(
                        ps_main[h][:, :], sl, a_slice(k, h),
                        start=(k == 0), stop=(k == kt - 1),
                    )
                epilogue(h)

    nc.compile()
    return nc


_CACHE = {}


def _get_nc():
    if "nc" not in _CACHE:
        _CACHE["nc"] = _build_nc()
    return _CACHE["nc"]


def _prep_in_maps(H, A, W, b):
    import ml_dtypes

    bf16 = ml_dtypes.bfloat16
    H = np.asarray(H, dtype=np.float32)
    A = np.asarray(A, dtype=np.float32)
    W = np.asarray(W, dtype=np.float32)
    b = np.asarray(b, dtype=np.float32)
    ht = np.ascontiguousarray(H.T.astype(bf16))
    wt = np.ascontiguousarray(W.T.astype(bf16))
    bias = np.ascontiguousarray(b.reshape(1, -1).astype(bf16))
    idx = np.arange(RPC)
    maps = []
    for i in range(NCORES):
        rows = slice(i * RPC, (i + 1) * RPC)
        Asl = A[rows, :].copy()
        Asl[idx, i * RPC + idx] += 1.0          # fold in A + I (0/1/2: exact)
        # pre-tile (A+I)[rows].T -> [128, kt*rpc], chunks of 8 k-tiles with
        # h-major columns: col = c*8192 + h*4096 + (k%8)*512 + i
        at = (Asl.T.reshape(8, 8, P, 2, 512)    # [c, kl, p, h, i]
              .transpose(2, 0, 3, 1, 4).reshape(P, -1))
        maps.append({
            "at": np.ascontiguousarray(at.astype(ml_dtypes.float8_e4m3)),
            "ht": ht,
            "wt": wt,
            "bias": bias,
        })
    return maps


def run(H, A, W, b, trace=False):
    from concourse import bass_utils

    nc = _get_nc()
    res = bass_utils.run_bass_kernel_spmd(
        nc, _prep_in_maps(H, A, W, b), core_ids=list(range(NCORES)),
        trace=trace,
    )
    Y = np.concatenate(
        [np.asarray(res.results[i]["out"]).T for i in range(NCORES)], axis=0
    )
    return np.ascontiguousarray(Y, dtype=np.float32), res


def kernel(H, A, W, b):
    return run(H, A, W, b)[0]


# revision 40
# speedup vs baseline: 1.1422x; 1.1165x over previous
"""Trainium2 Bass kernel: basic GCN layer, row-parallel over 8 NeuronCores.

    Y = relu( D^-1/2 (A + I) D^-1/2 (H @ W.T + b) ),  D = (A + I).sum(axis=1)

Sharding: core i owns output rows [i*1024, (i+1)*1024).  Each core receives
(A + I)[rows, :].T pre-tiled host-side into [128, 64*1024] fp8 so every DMA
descriptor moves an 8 KiB contiguous line (A+I is 0/1/2 — fp8 is lossless).
A stays fp8 in SBUF; matmuls use it as the moving operand against bf16
stationaries (mixed-dtype matmul is legal on TRN2).

Schedule per core (the first collective cannot start before the ~40-65 us
entry barrier + ~11 us ncfw setup, so everything before it is free time):
  - A loads in 8 chunks balanced over the gpsimd/sync/scalar DMA rings
    (~105 GB/s each); the PE runs the row-sum matmuls paced with the
    arriving chunks, as fp8 DoubleRow matmuls: adjacent k-tile pairs ride
    the two weight planes (all-ones), streaming A at 2 elements/cycle.
  - H.T follows on the same rings; HW = H @ W.T + b is computed in
    4-k-tile batches per PSUM bank, copied to SBUF by the scalar engine
    (keeps the vector queue clear and avoids per-tile ping-pong stalls).
  - Row sums go out in ONE 8-rank AllGather (4 KiB), triggered right
    after the row-sum matmuls finish (~1.5 us past the last A byte).
  - When the AG lands: gathered sums -> PE transpose -> dinv = 1/sqrt on
    [128, 64]; the 64 per-k-tile scales of HW alternate vector/scalar,
    racing ahead of the main matmuls (X^T A-tile into two PSUM halves);
    epilogue relu(dl * main); DMA out per half on separate rings.
"""

import os
import sys

import numpy as np

for _p in ("/opt/trn_rl_repo", "/root/.axon_site/_ro/trn_rl_repo"):
    if _p not in sys.path and os.path.isdir(_p):
        sys.path.insert(0, _p)

N = 8192        # nodes
NCORES = 8
RPC = N // NCORES  # rows per core (1024)
P = 128         # partitions / tile edge
F = 128         # feature dim (in == out)


def _build_nc(n=8192, rpc=1024, f=128, ncores=8):
    import concourse.bass as bass  # noqa: F401
    import concourse.mybir as mybir
    from concourse import bacc, tile
    from concourse.masks import make_identity

    dt = mybir.dt
    f32, bf, f8 = dt.float32, dt.bfloat16, dt.float8e4

    P = 128
    kt = n // P                 # contraction tiles (64)
    NCH = 8                     # A DMA chunks
    kpc = kt // NCH             # k-tiles per chunk (8)
    RC = 512                    # PSUM half width
    NRC = rpc // RC             # 2 halves
    KB = 4                      # k-tiles batched per hw-pre PSUM bank
    NB = kt // KB               # 16 hw-pre batches

    nc = bacc.Bacc("TRN2", num_devices=ncores)

    at = nc.dram_tensor("at", [P, kt * rpc], f8, kind="ExternalInput")   # (A+I)[rows].T pre-tiled
    ht = nc.dram_tensor("ht", [f, n], bf, kind="ExternalInput")          # H.T
    wt = nc.dram_tensor("wt", [f, f], bf, kind="ExternalInput")          # W.T
    bias = nc.dram_tensor("bias", [1, f], bf, kind="ExternalInput")      # b
    out = nc.dram_tensor("out", [f, rpc], bf, kind="ExternalOutput")     # Y[rows].T

    with tile.TileContext(nc) as tc:
        with (
            tc.tile_pool(name="const", bufs=1) as cpool,
            tc.tile_pool(name="abuf", bufs=1) as apool,
            tc.tile_pool(name="work", bufs=1) as wpool,
            tc.tile_pool(name="pshw", bufs=3, space="PSUM") as pshw,
            tc.tile_pool(name="psbig", bufs=1, space="PSUM") as psbig,
            tc.tile_pool(name="dram", bufs=1, space="DRAM") as dpool,
        ):
            # ---- constants / small inputs ----
            wt_sb = cpool.tile([f, f], bf, tag="wt", name="wt_sb")
            bias_sb = cpool.tile([1, f], bf, tag="bias", name="bias_sb")
            ones2 = cpool.tile([P, 2 * P], f8, tag="ones2", name="ones2")
            ones_r = cpool.tile([1, P], bf, tag="onesr", name="ones_r")
            ident = cpool.tile([P, P], f32, tag="ident", name="ident")
            nc.vector.memset(ones2[:], 1.0)
            nc.vector.memset(ones_r[:], 1.0)
            make_identity(nc, ident[:])
            nc.scalar.dma_start(wt_sb[:], wt[:])
            nc.scalar.dma_start(bias_sb[:], bias[:])

            # ---- A over three DMA rings in few big transfers ----
            # gp: chunks 0-1, 2 | sync: chunks 3-4, 5 | scalar: 6, 7
            # (scalar also carries the tiny ci upload so the AG trigger
            # never queues behind H.T).
            a_all = apool.tile([P, kt * rpc], f8, tag="a", name="a_all")
            ht_all = cpool.tile([f, n], bf, tag="ht", name="ht_all")
            W8 = kpc * rpc                      # bytes per chunk row-block

            def a_dma(eng, c, parts=1):
                for q in range(parts):
                    lo = c * W8 + q * W8 // parts
                    hi = c * W8 + (q + 1) * W8 // parts
                    eng.dma_start(a_all[:, lo:hi], at[:, lo:hi])

            ring_of = [nc.gpsimd, nc.sync, nc.scalar, nc.gpsimd,
                       nc.sync, nc.scalar, nc.gpsimd, nc.sync]
            for c in range(NCH):
                a_dma(ring_of[c], c, parts=2 if c < 3 else 1)
            ht_ring = [nc.gpsimd, nc.sync, nc.scalar, nc.gpsimd]
            for c in range(4):
                ht_ring[c].dma_start(
                    ht_all[:, c * 2 * rpc:(c + 1) * 2 * rpc],
                    ht[:, c * 2 * rpc:(c + 1) * 2 * rpc])
            a_ch = [a_all[:, c * W8:(c + 1) * W8] for c in range(NCH)]
            ht_ch = [ht_all[:, c * rpc:(c + 1) * rpc] for c in range(NCH)]

            # chunk layout is h-major: col = h*(kpc*RC) + (k%kpc)*RC + i
            def a_slice(k, h):
                base = h * kpc * RC + (k % kpc) * RC
                return a_ch[k // kpc][:, base:base + RC]

            # ---- row sums, paced with the A chunks ----
            # fp8 DoubleRow: adjacent k-tile pairs ride the two weight
            # planes (all-ones), streaming A at 2 elements/cycle.
            ps_rs = [psbig.tile([P, RC], f32, tag=f"rs{h}", name=f"rs{h}")
                     for h in range(NRC)]
            ones2_ap = ones2[:].rearrange("p (o m) -> p o m", o=2)
            ppc = kpc // 2                      # pairs per chunk
            chunk_order = list(range(NCH))      # matches 3-ring arrival
            for ci_, c in enumerate(chunk_order):
                for tl in range(ppc):
                    for h in range(NRC):
                        base = h * kpc * RC + tl * 2 * RC
                        pair = a_ch[c][:, base:base + 2 * RC].rearrange(
                            "p (o i) -> p o i", o=2)
                        nc.tensor.matmul(
                            ps_rs[h][:, :], ones2_ap, pair,
                            start=(ci_ == 0 and tl == 0),
                            stop=(ci_ == NCH - 1 and tl == ppc - 1),
                            perf_mode=mybir.MatmulPerfMode.DoubleRow)

            # row sums -> SBUF (vector engine) -> DRAM -> AllGather
            rs_sb = wpool.tile([1, rpc], f32, tag="rs_sb", name="rs_sb")
            for h in range(NRC):
                nc.vector.tensor_copy(rs_sb[0:1, h * RC:(h + 1) * RC],
                                      ps_rs[h][0:1, :])
            ci = dpool.tile([1, rpc], f32, tag="ccin", name="cc_in")
            co = dpool.tile([ncores, rpc], f32, tag="ccout", name="cc_out",
                            addr_space="Shared")
            nc.gpsimd.dma_start(ci[:], rs_sb[:])
            nc.gpsimd.collective_compute(
                "AllGather", mybir.AluOpType.bypass,
                replica_groups=[list(range(ncores))],
                ins=[ci.opt()], outs=[co.opt()],
            )

            # ---- HW = H @ W.T + b, 4 k-tiles per PSUM bank ----
            hw_sb = wpool.tile([P, n], bf, tag="hw", name="hw_sb")
            for g in range(NB):
                ps4 = pshw.tile([P, KB * f], f32, tag="hw4", name=f"hw4_{g}")
                for m in range(KB):
                    k = g * KB + m
                    sl = ps4[:, m * f:(m + 1) * f]
                    nc.tensor.matmul(sl,
                                     ht_ch[k // kpc][:, (k % kpc) * P:
                                                     (k % kpc + 1) * P],
                                     wt_sb[:, :], start=True, stop=False)
                    nc.tensor.matmul(sl, ones_r[0:1, 0:P],
                                     bias_sb[0:1, :], start=False, stop=True)
                nc.scalar.copy(hw_sb[:, g * KB * f:(g + 1) * KB * f], ps4[:, :])

            # ---- dl broadcast for the epilogue: dlb[p, i] = D_i^-1/2 ----
            # computed during the AG wait (only needs the local sums); its
            # sqrt also absorbs the scalar engine's COPY->SQRT activation
            # table reload so the dinv sqrt below doesn't pay it.
            dlb = wpool.tile([P, rpc], f32, tag="dlb", name="dlb")
            nc.sync.dma_start(
                dlb[:].rearrange("p (o r) -> p o r", o=1),
                ci[0:1, :].partition_broadcast(P),
            )
            nc.scalar.sqrt(dlb[:, :], dlb[:, :])
            nc.vector.reciprocal(dlb[:, :], dlb[:, :])

            # ---- gathered sums -> dinv[p, k] = 1/sqrt(s[128k + p]) ----
            # two parallel half-DMAs shave latency off the post-AG chain
            rs2d = wpool.tile([kt, P], f32, tag="rs2d", name="rs2d")
            co2d = co[:].rearrange("g (m p) -> (g m) p", p=P)
            nc.sync.dma_start(rs2d[0:kt // 2, :], co2d[0:kt // 2, :])
            nc.gpsimd.dma_start(rs2d[kt // 2:kt, :], co2d[kt // 2:kt, :])
            ps_t = pshw.tile([P, kt], f32, tag="hw4", name="ps_t")
            nc.tensor.transpose(ps_t[:, :], rs2d[:, :], ident[0:kt, 0:kt])
            dinv = wpool.tile([P, kt], f32, tag="dinv", name="dinv")
            nc.scalar.sqrt(dinv[:, :], ps_t[:, :])
            nc.vector.reciprocal(dinv[:, :], dinv[:, :])

            # ---- scale + main matmuls: ps_main[h] += (dinv_k*HW_k)^T A_k ----
            # h-major: ps_main[0] finishes halfway so its epilogue and
            # output DMA overlap the h=1 matmuls.  The per-k scales
            # alternate vector/scalar (h=0 pass only) so either engine
            # only has to keep half pace with the PE.
            ps_main = [psbig.tile([f, RC], f32, tag=f"main{h}", name=f"main{h}")
                       for h in range(NRC)]
            y_sb = wpool.tile([f, rpc], bf, tag="y", name="y_sb")
            QC = RC // 2

            def epilogue(h):
                for m in range(2):
                    lo = h * RC + m * QC
                    sl = y_sb[:, lo:lo + QC]
                    nc.vector.tensor_mul(sl,
                                         ps_main[h][:, m * QC:(m + 1) * QC],
                                         dlb[:, lo:lo + QC])
                    nc.vector.tensor_scalar_max(sl, sl, 0.0)
                    eng = nc.gpsimd if m == 0 else nc.sync
                    eng.dma_start(out[:, lo:lo + QC], sl)

            for h in range(NRC):
                for k in range(kt):
                    sl = hw_sb[:, k * f:(k + 1) * f]
                    if h == 0:
                        if k % 2 == 0:
                            nc.vector.tensor_scalar_mul(sl, sl,
                                                        dinv[:, k:k + 1])
                        else:
                            nc.scalar.mul(sl, sl, dinv[:, k:k + 1])
                    nc.tensor.matmul(
                        ps_main[h][:, :], sl, a_slice(k, h),
                        start=(k == 0), stop=(k == kt - 1),
                    )
                epilogue(h)

    nc.compile()
    return nc


_CACHE = {}


def _get_nc():
    if "nc" not in _CACHE:
        _CACHE["nc"] = _build_nc()
    return _CACHE["nc"]


def _prep_in_maps(H, A, W, b):
    import ml_dtypes

    bf16 = ml_dtypes.bfloat16
    H = np.asarray(H, dtype=np.float32)
    A = np.asarray(A, dtype=np.float32)
    W = np.asarray(W, dtype=np.float32)
    b = np.asarray(b, dtype=np.float32)
    ht = np.ascontiguousarray(H.T.astype(bf16))
    wt = np.ascontiguousarray(W.T.astype(bf16))
    bias = np.ascontiguousarray(b.reshape(1, -1).astype(bf16))
    idx = np.arange(RPC)
    maps = []
    for i in range(NCORES):
        rows = slice(i * RPC, (i + 1) * RPC)
        Asl = A[rows, :].copy()
        Asl[idx, i * RPC + idx] += 1.0          # fold in A + I (0/1/2: exact)
        # pre-tile (A+I)[rows].T -> [128, kt*rpc], chunks of 8 k-tiles with
        # h-major columns: col = c*8192 + h*4096 + (k%8)*512 + i
        at = (Asl.T.reshape(8, 8, P, 2, 512)    # [c, kl, p, h, i]
              .transpose(2, 0, 3, 1, 4).reshape(P, -1))
        maps.append({
            "at": np.ascontiguousarray(at.astype(ml_dtypes.float8_e4m3)),
            "ht": ht,
            "wt": wt,
            "bias": bias,
        })
    return maps


def run(H, A, W, b, trace=False):
    from concourse import bass_utils

    nc = _get_nc()
    res = bass_utils.run_bass_kernel_spmd(
        nc, _prep_in_maps(H, A, W, b), core_ids=list(range(NCORES)),
        trace=trace,
    )
    Y = np.concatenate(
        [np.asarray(res.results[i]["out"]).T for i in range(NCORES)], axis=0
    )
    return np.ascontiguousarray(Y, dtype=np.float32), res


def kernel(H, A, W, b):
    return run(H, A, W, b)[0]


# revision 41
# speedup vs baseline: 1.1979x; 1.0488x over previous
"""Trainium2 Bass kernel: basic GCN layer, row-parallel over 8 NeuronCores.

    Y = relu( D^-1/2 (A + I) D^-1/2 (H @ W.T + b) ),  D = (A + I).sum(axis=1)

Sharding: core i owns output rows [i*1024, (i+1)*1024).  Each core receives
(A + I)[rows, :].T pre-tiled host-side into [128, 64*1024] fp8 so every DMA
descriptor moves an 8 KiB contiguous line (A+I is 0/1/2 — fp8 is lossless).
A stays fp8 in SBUF; matmuls use it as the moving operand against bf16
stationaries (mixed-dtype matmul is legal on TRN2).

Schedule per core (the first collective cannot start before the ~40-65 us
entry barrier + ~11 us ncfw setup, so everything before it is free time):
  - A loads in 8 chunks balanced over the gpsimd/sync/scalar DMA rings
    (~105 GB/s each); the PE runs the row-sum matmuls paced with the
    arriving chunks, as fp8 DoubleRow matmuls: adjacent k-tile pairs ride
    the two weight planes (all-ones), streaming A at 2 elements/cycle.
  - H.T follows on the same rings; HW = H @ W.T + b is computed in
    4-k-tile batches per PSUM bank, copied to SBUF by the scalar engine
    (keeps the vector queue clear and avoids per-tile ping-pong stalls).
  - Row sums go out in ONE 8-rank AllGather (4 KiB), triggered right
    after the row-sum matmuls finish (~1.5 us past the last A byte).
  - When the AG lands: gathered sums -> PE transpose -> dinv = 1/sqrt on
    [128, 64]; the 64 per-k-tile scales of HW alternate vector/scalar,
    racing ahead of the main matmuls (X^T A-tile into two PSUM halves);
    epilogue relu(dl * main); DMA out per half on separate rings.
"""

import os
import sys

import numpy as np

for _p in ("/opt/trn_rl_repo", "/root/.axon_site/_ro/trn_rl_repo"):
    if _p not in sys.path and os.path.isdir(_p):
        sys.path.insert(0, _p)

N = 8192        # nodes
NCORES = 8
RPC = N // NCORES  # rows per core (1024)
P = 128         # partitions / tile edge
F = 128         # feature dim (in == out)


def _build_nc(n=8192, rpc=1024, f=128, ncores=8):
    import concourse.bass as bass  # noqa: F401
    import concourse.mybir as mybir
    from concourse import bacc, tile
    from concourse.masks import make_identity

    dt = mybir.dt
    f32, bf, f8 = dt.float32, dt.bfloat16, dt.float8e4

    P = 128
    kt = n // P                 # contraction tiles (64)
    NCH = 8                     # A DMA chunks
    kpc = kt // NCH             # k-tiles per chunk (8)
    RC = 512                    # PSUM half width
    NRC = rpc // RC             # 2 halves
    KB = 4                      # k-tiles batched per hw-pre PSUM bank
    NB = kt // KB               # 16 hw-pre batches

    nc = bacc.Bacc("TRN2", num_devices=ncores)

    at = nc.dram_tensor("at", [P, kt * rpc], f8, kind="ExternalInput")   # (A+I)[rows].T pre-tiled
    ht = nc.dram_tensor("ht", [f, n], bf, kind="ExternalInput")          # H.T
    wt = nc.dram_tensor("wt", [f, f], bf, kind="ExternalInput")          # W.T
    bias = nc.dram_tensor("bias", [1, f], bf, kind="ExternalInput")      # b
    out = nc.dram_tensor("out", [f, rpc], bf, kind="ExternalOutput")     # Y[rows].T

    with tile.TileContext(nc) as tc:
        with (
            tc.tile_pool(name="const", bufs=1) as cpool,
            tc.tile_pool(name="abuf", bufs=1) as apool,
            tc.tile_pool(name="work", bufs=1) as wpool,
            tc.tile_pool(name="pshw", bufs=3, space="PSUM") as pshw,
            tc.tile_pool(name="psbig", bufs=1, space="PSUM") as psbig,
            tc.tile_pool(name="dram", bufs=1, space="DRAM") as dpool,
        ):
            # ---- constants / small inputs ----
            wt_sb = cpool.tile([f, f], bf, tag="wt", name="wt_sb")
            bias_sb = cpool.tile([1, f], bf, tag="bias", name="bias_sb")
            ones2 = cpool.tile([P, 2 * P], f8, tag="ones2", name="ones2")
            ones_r = cpool.tile([1, P], bf, tag="onesr", name="ones_r")
            ident = cpool.tile([P, P], f32, tag="ident", name="ident")
            nc.vector.memset(ones2[:], 1.0)
            nc.vector.memset(ones_r[:], 1.0)
            make_identity(nc, ident[:])
            nc.scalar.dma_start(wt_sb[:], wt[:])
            nc.scalar.dma_start(bias_sb[:], bias[:])

            # ---- A over three DMA rings in few big transfers ----
            # gp: chunks 0-1, 2 | sync: chunks 3-4, 5 | scalar: 6, 7
            # (scalar also carries the tiny ci upload so the AG trigger
            # never queues behind H.T).
            a_all = apool.tile([P, kt * rpc], f8, tag="a", name="a_all")
            ht_all = cpool.tile([f, n], bf, tag="ht", name="ht_all")
            W8 = kpc * rpc                      # bytes per chunk row-block

            def a_dma(eng, c, parts=1):
                for q in range(parts):
                    lo = c * W8 + q * W8 // parts
                    hi = c * W8 + (q + 1) * W8 // parts
                    eng.dma_start(a_all[:, lo:hi], at[:, lo:hi])

            ring_of = [nc.gpsimd, nc.sync, nc.scalar, nc.gpsimd,
                       nc.sync, nc.scalar, nc.gpsimd, nc.sync]
            for c in range(NCH):
                a_dma(ring_of[c], c, parts=2 if c < 3 else 1)
            ht_ring = [nc.gpsimd, nc.sync, nc.scalar, nc.scalar]
            for c in range(4):
                ht_ring[c].dma_start(
                    ht_all[:, c * 2 * rpc:(c + 1) * 2 * rpc],
                    ht[:, c * 2 * rpc:(c + 1) * 2 * rpc])
            a_ch = [a_all[:, c * W8:(c + 1) * W8] for c in range(NCH)]
            ht_ch = [ht_all[:, c * rpc:(c + 1) * rpc] for c in range(NCH)]

            # chunk layout is h-major: col = h*(kpc*RC) + (k%kpc)*RC + i
            def a_slice(k, h):
                base = h * kpc * RC + (k % kpc) * RC
                return a_ch[k // kpc][:, base:base + RC]

            # ---- row sums, paced with the A chunks ----
            # fp8 DoubleRow: adjacent k-tile pairs ride the two weight
            # planes (all-ones), streaming A at 2 elements/cycle.
            ps_rs = [psbig.tile([P, RC], f32, tag=f"rs{h}", name=f"rs{h}")
                     for h in range(NRC)]
            ones2_ap = ones2[:].rearrange("p (o m) -> p o m", o=2)
            ppc = kpc // 2                      # pairs per chunk
            chunk_order = list(range(NCH))      # matches 3-ring arrival
            for ci_, c in enumerate(chunk_order):
                for tl in range(ppc):
                    for h in range(NRC):
                        base = h * kpc * RC + tl * 2 * RC
                        pair = a_ch[c][:, base:base + 2 * RC].rearrange(
                            "p (o i) -> p o i", o=2)
                        nc.tensor.matmul(
                            ps_rs[h][:, :], ones2_ap, pair,
                            start=(ci_ == 0 and tl == 0),
                            stop=(ci_ == NCH - 1 and tl == ppc - 1),
                            perf_mode=mybir.MatmulPerfMode.DoubleRow)

            # row sums -> SBUF (vector engine) -> DRAM -> AllGather
            rs_sb = wpool.tile([1, rpc], f32, tag="rs_sb", name="rs_sb")
            for h in range(NRC):
                nc.vector.tensor_copy(rs_sb[0:1, h * RC:(h + 1) * RC],
                                      ps_rs[h][0:1, :])
            ci = dpool.tile([1, rpc], f32, tag="ccin", name="cc_in")
            co = dpool.tile([ncores, rpc], f32, tag="ccout", name="cc_out",
                            addr_space="Shared")
            nc.gpsimd.dma_start(ci[:], rs_sb[:])
            nc.gpsimd.collective_compute(
                "AllGather", mybir.AluOpType.bypass,
                replica_groups=[list(range(ncores))],
                ins=[ci.opt()], outs=[co.opt()],
            )

            # ---- HW = H @ W.T + b, 4 k-tiles per PSUM bank ----
            hw_sb = wpool.tile([P, n], bf, tag="hw", name="hw_sb")
            for g in range(NB):
                ps4 = pshw.tile([P, KB * f], f32, tag="hw4", name=f"hw4_{g}")
                for m in range(KB):
                    k = g * KB + m
                    sl = ps4[:, m * f:(m + 1) * f]
                    nc.tensor.matmul(sl,
                                     ht_ch[k // kpc][:, (k % kpc) * P:
                                                     (k % kpc + 1) * P],
                                     wt_sb[:, :], start=True, stop=False)
                    nc.tensor.matmul(sl, ones_r[0:1, 0:P],
                                     bias_sb[0:1, :], start=False, stop=True)
                nc.scalar.copy(hw_sb[:, g * KB * f:(g + 1) * KB * f], ps4[:, :])

            # ---- dl broadcast for the epilogue: dlb[p, i] = D_i^-1/2 ----
            # computed during the AG wait (only needs the local sums); its
            # sqrt also absorbs the scalar engine's COPY->SQRT activation
            # table reload so the dinv sqrt below doesn't pay it.
            dlb = wpool.tile([P, rpc], f32, tag="dlb", name="dlb")
            nc.sync.dma_start(
                dlb[:].rearrange("p (o r) -> p o r", o=1),
                ci[0:1, :].partition_broadcast(P),
            )
            nc.scalar.sqrt(dlb[:, :], dlb[:, :])
            nc.vector.reciprocal(dlb[:, :], dlb[:, :])

            # ---- gathered sums -> dinv[p, k] = 1/sqrt(s[128k + p]) ----
            # two parallel half-DMAs shave latency off the post-AG chain
            rs2d = wpool.tile([kt, P], f32, tag="rs2d", name="rs2d")
            co2d = co[:].rearrange("g (m p) -> (g m) p", p=P)
            nc.sync.dma_start(rs2d[0:kt // 2, :], co2d[0:kt // 2, :])
            nc.gpsimd.dma_start(rs2d[kt // 2:kt, :], co2d[kt // 2:kt, :])
            ps_t = pshw.tile([P, kt], f32, tag="hw4", name="ps_t")
            nc.tensor.transpose(ps_t[:, :], rs2d[:, :], ident[0:kt, 0:kt])
            dinv = wpool.tile([P, kt], f32, tag="dinv", name="dinv")
            nc.scalar.sqrt(dinv[:, :], ps_t[:, :])
            nc.vector.reciprocal(dinv[:, :], dinv[:, :])

            # ---- scale + main matmuls: ps_main[h] += (dinv_k*HW_k)^T A_k ----
            # h-major: ps_main[0] finishes halfway so its epilogue and
            # output DMA overlap the h=1 matmuls.  The per-k scales
            # alternate vector/scalar (h=0 pass only) so either engine
            # only has to keep half pace with the PE.
            ps_main = [psbig.tile([f, RC], f32, tag=f"main{h}", name=f"main{h}")
                       for h in range(NRC)]
            y_sb = wpool.tile([f, rpc], bf, tag="y", name="y_sb")
            QC = RC // 2

            def epilogue(h):
                for m in range(2):
                    lo = h * RC + m * QC
                    sl = y_sb[:, lo:lo + QC]
                    nc.vector.tensor_mul(sl,
                                         ps_main[h][:, m * QC:(m + 1) * QC],
                                         dlb[:, lo:lo + QC])
                    nc.vector.tensor_scalar_max(sl, sl, 0.0)
                    eng = nc.gpsimd if m == 0 else nc.sync
                    eng.dma_start(out[:, lo:lo + QC], sl)

            for h in range(NRC):
                for k in range(kt):
                    sl = hw_sb[:, k * f:(k + 1) * f]
                    if h == 0:
                        if k % 2 == 0:
                            nc.vector.tensor_scalar_mul(sl, sl,
                                                        dinv[:, k:k + 1])
                        else:
                            nc.scalar.mul(sl, sl, dinv[:, k:k + 1])
                    nc.tensor.matmul(
                        ps_main[h][:, :], sl, a_slice(k, h),
                        start=(k == 0), stop=(k == kt - 1),
                    )
                epilogue(h)

    nc.compile()
    return nc


_CACHE = {}


def _get_nc():
    if "nc" not in _CACHE:
        _CACHE["nc"] = _build_nc()
    return _CACHE["nc"]


def _prep_in_maps(H, A, W, b):
    import ml_dtypes

    bf16 = ml_dtypes.bfloat16
    H = np.asarray(H, dtype=np.float32)
    A = np.asarray(A, dtype=np.float32)
    W = np.asarray(W, dtype=np.float32)
    b = np.asarray(b, dtype=np.float32)
    ht = np.ascontiguousarray(H.T.astype(bf16))
    wt = np.ascontiguousarray(W.T.astype(bf16))
    bias = np.ascontiguousarray(b.reshape(1, -1).astype(bf16))
    idx = np.arange(RPC)
    maps = []
    for i in range(NCORES):
        rows = slice(i * RPC, (i + 1) * RPC)
        Asl = A[rows, :].copy()
        Asl[idx, i * RPC + idx] += 1.0          # fold in A + I (0/1/2: exact)
        # pre-tile (A+I)[rows].T -> [128, kt*rpc], chunks of 8 k-tiles with
        # h-major columns: col = c*8192 + h*4096 + (k%8)*512 + i
        at = (Asl.T.reshape(8, 8, P, 2, 512)    # [c, kl, p, h, i]
              .transpose(2, 0, 3, 1, 4).reshape(P, -1))
        maps.append({
            "at": np.ascontiguousarray(at.astype(ml_dtypes.float8_e4m3)),
            "ht": ht,
            "wt": wt,
            "bias": bias,
        })
    return maps


def run(H, A, W, b, trace=False):
    from concourse import bass_utils

    nc = _get_nc()
    res = bass_utils.run_bass_kernel_spmd(
        nc, _prep_in_maps(H, A, W, b), core_ids=list(range(NCORES)),
        trace=trace,
    )
    Y = np.concatenate(
        [np.asarray(res.results[i]["out"]).T for i in range(NCORES)], axis=0
    )
    return np.ascontiguousarray(Y, dtype=np.float32), res


def kernel(H, A, W, b):
    return run(H, A, W, b)[0]
